# revision 1
# baseline (speedup 1.0000x reference)
"""Trainium2 Bass kernel for nn_CrossDomainAttention (B=4, C=128, D*H*W=131072).

Math reduction (host folds the query chain):
  scores[b,h,n] = scale * qh[b,h] . (wk_h @ x_n + bk_h)  ==  a[b,h] . x_n + const
  softmax is shift-invariant -> drop the const.  attn = softmax(a.x)
  ctx[b, h*32+d] = wv[h*32+d,:] @ (sum_n attn_n x_n) + bv       (sum attn = 1)
  out = wo @ ctx + bo ; ln = LayerNorm(out) ; result = x + ln[:, None]

Device (8 cores SPMD; core r handles batch r//2, token half r%2 = 65536 tokens):
  pass A (bf16): per 128-token block compute logitsT = x_blk.T @ a and
    xT = x_blk.T @ I on PE; w' = exp(logits)-1 (or w'=logits, Taylor mode);
    accumulate [sum w' x | sum w'; sum x | count] into one PSUM tile via
    matmuls with an appended ones row/column.
  AllGather 5x129 partials -> every core redundantly combines its batch,
    computes ctx/out/LayerNorm -> ln (128,1).
  pass B (fp32): re-stream x, tensor_scalar add ln per partition, store.
"""

import math
import os
import sys
from contextlib import ExitStack

import numpy as np

if "/opt/trn_rl_repo" not in sys.path:
    sys.path.insert(0, "/opt/trn_rl_repo")

import ml_dtypes

import concourse.bass as bass
import concourse.mybir as mybir
import concourse.tile as tile
from concourse.bass_utils import run_bass_kernel_spmd


def _legalize_sync_waits(bir_json: bytes) -> bytes:
    """This toolchain's walrus supports one sync-wait slot per instruction
    (ISA EVENTS struct). Tile emits instructions with several waits; split
    the extras onto same-engine NoOps placed immediately before."""
    import orjson

    d = orjson.loads(bir_json)
    ctr = 0
    for f in d.get("functions", []):
        for bb in f.get("blocks", []):
            new = []
            changed = False
            for inst in bb.get("instructions", []):
                si = inst.get("sync_info")
                waits = (si or {}).get("on_wait") or []
                if len(waits) > 1:
                    changed = True
                    for w in waits[:-1]:
                        ctr += 1
                        nop = {
                            "engine": inst["engine"],
                            "ins": [],
                            "outs": [],
                            "name": f"legwait-{ctr}",
                            "opcode": "NoOp",
                            "sync_info": {"on_update": [], "on_wait": [w]},
                        }
                        if "debug" in inst:
                            nop["debug"] = inst["debug"]
                        new.append(nop)
                    si["on_wait"] = [waits[-1]]
                new.append(inst)
            if changed:
                bb["instructions"] = new
    return orjson.dumps(d)


def _install_sync_wait_patch():
    import concourse.bass_utils as bu
    import concourse.bass2jax as b2j

    if getattr(bu, "_sync_wait_patch", False):
        return
    orig = bu.compile_bir_kernel

    def patched(bir_json, tmpdir, neff_name="file.neff"):
        return orig(_legalize_sync_waits(bytes(bir_json)), tmpdir, neff_name)

    bu.compile_bir_kernel = patched
    bu._sync_wait_patch = True
    if getattr(b2j, "compile_bir_kernel", None) is orig:
        b2j.compile_bir_kernel = patched


_install_sync_wait_patch()

F32 = mybir.dt.float32
BF16 = mybir.dt.bfloat16

B = 4
C = 128          # embed dim == channel dim
NH = 4           # heads
HD = 32
N_FULL = 32 * 64 * 64   # 131072 tokens per batch
N_CORES = 8
TOK = N_FULL // 2       # tokens per core (65536)
SCALE = 1.0 / math.sqrt(HD)
LN_EPS = 1e-5

# knobs
TILE_W = 4096            # tokens per DMA tile (pass A and pass B)
CHUNK = 512              # tokens per inner chunk (psum xT batch)
BLK = 128                # tokens per PE block

# module-level controls for the test harness
TRACE = False
LAST_EXEC_NS = None
LAST_RESULTS = None
LAST_IN_MAPS = None
LAST_TAYLOR = True


def _build_trivial_nc():
    """Passthrough kernel with identical I/O: out = copy(xf). Used by the
    test harness to subtract transfer/dispatch overhead when timing."""
    nc = bass.Bass("TRN2", target_bir_lowering=False, debug=False,
                   num_devices=N_CORES)
    xf_d = nc.dram_tensor("xf", [C, TOK], F32, kind="ExternalInput")
    nc.dram_tensor("xb", [C, TOK], BF16, kind="ExternalInput")
    nc.dram_tensor("a_q", [C, NH], BF16, kind="ExternalInput")
    nc.dram_tensor("ident", [128, 128], BF16, kind="ExternalInput")
    nc.dram_tensor("identf", [NH, NH], F32, kind="ExternalInput")
    nc.dram_tensor("ones_f", [128, 1], F32, kind="ExternalInput")
    nc.dram_tensor("ones_row", [1, 128], F32, kind="ExternalInput")
    nc.dram_tensor("sel", [5 * N_CORES, NH], F32, kind="ExternalInput")
    nc.dram_tensor("wvt", [C, C], F32, kind="ExternalInput")
    nc.dram_tensor("wot", [C, C], F32, kind="ExternalInput")
    nc.dram_tensor("vecs", [C, 4], F32, kind="ExternalInput")
    out_d = nc.dram_tensor("out", [C, TOK], F32, kind="ExternalOutput")
    with tile.TileContext(nc) as tc:
        with tc.tile_pool(name="p", bufs=2) as p:
            t0 = p.tile([C, TILE_W], F32)
            nc.sync.dma_start(out=t0, in_=xf_d[:, 0:TILE_W])
            nc.sync.dma_start(out=out_d[:, 0:TILE_W], in_=t0)
    return nc


def _build_nc(tok=TOK, tile_w=TILE_W, taylor=True):
    """Emit the SPMD program for one core (same program on all 8)."""
    nt = tok // tile_w
    chunks_per_tile = tile_w // CHUNK
    blocks_per_chunk = CHUNK // BLK
    total_blocks = tok // BLK

    nc = bass.Bass("TRN2", target_bir_lowering=False, debug=False,
                   num_devices=N_CORES)

    # ---- DRAM I/O ----
    xb_d = nc.dram_tensor("xb", [C, tok], BF16, kind="ExternalInput")
    xf_d = nc.dram_tensor("xf", [C, tok], F32, kind="ExternalInput")
    a_d = nc.dram_tensor("a_q", [C, NH], BF16, kind="ExternalInput")
    ident_d = nc.dram_tensor("ident", [128, 128], BF16, kind="ExternalInput")
    identf_d = nc.dram_tensor("identf", [NH, NH], F32, kind="ExternalInput")
    onesf_d = nc.dram_tensor("ones_f", [128, 1], F32, kind="ExternalInput")
    onesrow_d = nc.dram_tensor("ones_row", [1, 128], F32, kind="ExternalInput")
    sel_d = nc.dram_tensor("sel", [5 * N_CORES, NH], F32, kind="ExternalInput")
    wvt_d = nc.dram_tensor("wvt", [C, C], F32, kind="ExternalInput")
    wot_d = nc.dram_tensor("wot", [C, C], F32, kind="ExternalInput")
    vecs_d = nc.dram_tensor("vecs", [C, 4], F32, kind="ExternalInput")
    out_d = nc.dram_tensor("out", [C, tok], F32, kind="ExternalOutput")

    with tile.TileContext(nc) as tc, ExitStack() as stack:
        consts = stack.enter_context(tc.tile_pool(name="consts", bufs=1))
        accp = stack.enter_context(
            tc.tile_pool(name="acc", bufs=1, space="PSUM"))
        dramp = stack.enter_context(
            tc.tile_pool(name="dram", bufs=1, space="DRAM"))

        # constants into SBUF
        a_sb = consts.tile([C, NH], BF16)
        nc.sync.dma_start(out=a_sb, in_=a_d[:, :])
        ident_sb = consts.tile([128, 128], BF16)
        nc.sync.dma_start(out=ident_sb, in_=ident_d[:, :])
        identf_sb = consts.tile([NH, NH], F32)
        nc.sync.dma_start(out=identf_sb, in_=identf_d[:, :])
        onesf_sb = consts.tile([128, 1], F32)
        nc.sync.dma_start(out=onesf_sb, in_=onesf_d[:, :])
        onesrow_sb = consts.tile([1, 128], F32)
        nc.sync.dma_start(out=onesrow_sb, in_=onesrow_d[:, :])
        sel_sb = consts.tile([5 * N_CORES, NH], F32)
        nc.sync.dma_start(out=sel_sb, in_=sel_d[:, :])
        wvt_sb = consts.tile([C, C], F32)
        nc.sync.dma_start(out=wvt_sb, in_=wvt_d[:, :])
        wot_sb = consts.tile([C, C], F32)
        nc.sync.dma_start(out=wot_sb, in_=wot_d[:, :])
        vecs_sb = consts.tile([C, 4], F32)
        nc.sync.dma_start(out=vecs_sb, in_=vecs_d[:, :])
        eps_sb = consts.tile([1, 1], F32)
        nc.vector.memset(eps_sb, LN_EPS)
        zeros_sb = consts.tile([128, 1], F32)
        nc.vector.memset(zeros_sb, 0.0)

        # persistent psum accumulator: rows 0..3 = [sum w'x | sum w']_h,
        # row 4 = [sum x | count]
        s_acc = accp.tile([5, 129], F32)

        # collective bounce buffers (DRAM)
        cc_in = dramp.tile([5, 129], F32)
        cc_out = dramp.tile([5 * N_CORES, 129], F32, addr_space="Shared")

        # ---------------- pass A ----------------
        blk_idx = 0
        with tc.tile_pool(name="xbf", bufs=3) as xbf_p, \
             tc.tile_pool(name="lg", bufs=2, space="PSUM") as lg_p, \
             tc.tile_pool(name="xtp", bufs=2, space="PSUM") as xtp_p, \
             tc.tile_pool(name="wtile", bufs=3) as w_p, \
             tc.tile_pool(name="xts", bufs=3) as xts_p:
            for t in range(nt):
                xt = xbf_p.tile([C, tile_w], BF16)
                nc.sync.dma_start(out=xt, in_=xb_d[:, t * tile_w:(t + 1) * tile_w])
                for ch in range(chunks_per_tile):
                    base = ch * CHUNK
                    lg = lg_p.tile([128, 4 * blocks_per_chunk], F32)
                    xtp = xtp_p.tile([128, CHUNK], F32)
                    for j in range(blocks_per_chunk):
                        xblk = xt[:, base + j * BLK: base + (j + 1) * BLK]
                        nc.tensor.matmul(lg[:, j * 4:(j + 1) * 4], lhsT=xblk,
                                         rhs=a_sb, start=True, stop=True)
                        nc.tensor.matmul(xtp[:, j * BLK:(j + 1) * BLK],
                                         lhsT=xblk, rhs=ident_sb,
                                         start=True, stop=True)
                    # w' tile: (128, 5*blocks): cols j*5+0..3 = w', j*5+4 = 1
                    wt = w_p.tile([128, 5 * blocks_per_chunk], BF16)
                    wt3 = wt.rearrange("p (j f) -> p j f", f=5)
                    nc.vector.memset(wt3[:, :, 4:5], 1.0)
                    lg3 = lg.rearrange("p (j f) -> p j f", f=4)
                    if taylor:
                        # w' = logits  (exp(l)-1 to first order)
                        nc.vector.tensor_copy(out=wt3[:, :, 0:4], in_=lg3)
                    else:
                        escr = w_p.tile([128, 4 * blocks_per_chunk], F32,
                                        tag="escr")
                        nc.scalar.activation(out=escr, in_=lg,
                                             func=mybir.ActivationFunctionType.Exp,
                                             bias=zeros_sb)
                        es3 = escr.rearrange("p (j f) -> p j f", f=4)
                        nc.vector.tensor_scalar(
                            out=wt3[:, :, 0:4], in0=es3, scalar1=-1.0,
                            scalar2=None, op0=mybir.AluOpType.add)
                    # xts: (128, 129*blocks): per block 128 cols xT + ones col
                    xts = xts_p.tile([128, 129 * blocks_per_chunk], BF16)
                    xts3 = xts.rearrange("p (j f) -> p j f", f=129)
                    nc.vector.memset(xts3[:, :, 128:129], 1.0)
                    if ch % 2 == 0:
                        nc.vector.tensor_copy(out=xts3[:, :, 0:128],
                                              in_=xtp.rearrange(
                                                  "p (j f) -> p j f", f=BLK))
                    else:
                        nc.scalar.copy(out=xts3[:, :, 0:128],
                                       in_=xtp.rearrange(
                                           "p (j f) -> p j f", f=BLK))
                    for j in range(blocks_per_chunk):
                        first = blk_idx == 0
                        last = blk_idx == total_blocks - 1
                        blk_idx += 1
                        nc.tensor.matmul(
                            s_acc[:, :],
                            lhsT=wt[:, j * 5:(j + 1) * 5],
                            rhs=xts[:, j * 129:(j + 1) * 129],
                            start=first, stop=last)

        # ---------------- collective + epilogue ----------------
        s_sb = consts.tile([5, 129], F32)
        nc.vector.tensor_copy(out=s_sb, in_=s_acc[:, :])
        nc.sync.dma_start(out=cc_in[:, :], in_=s_sb[:, :])
        nc.gpsimd.collective_compute(
            "AllGather",
            mybir.AluOpType.bypass,
            replica_groups=[list(range(N_CORES))],
            ins=[cc_in[:, :].opt()],
            outs=[cc_out[:, :].opt()],
        )

        with tc.tile_pool(name="epi", bufs=1) as ep, \
             tc.tile_pool(name="epp", bufs=2, space="PSUM") as epp:
            g_sb = ep.tile([5 * N_CORES, 129], F32)
            nc.sync.dma_start(out=g_sb, in_=cc_out[:, :])
            # comb[h,:] = sum over my pair of (S'_h + S'_ones) rows
            comb = epp.tile([NH, 129], F32, tag="ep_ps")
            nc.tensor.matmul(comb, lhsT=sel_sb, rhs=g_sb, start=True, stop=True)
            inv = ep.tile([NH, 1], F32)
            nc.vector.reciprocal(inv, comb[:, 128:129])
            t_sb = ep.tile([NH, C], F32)
            nc.vector.tensor_scalar_mul(t_sb, comb[:, 0:128], inv)
            # transpose T (4,128) -> (128,4)
            tt_p = epp.tile([C, NH], F32, tag="ep_ps")
            nc.tensor.transpose(tt_p, t_sb, identf_sb)
            tt_sb = ep.tile([C, NH], F32)
            nc.vector.tensor_copy(tt_sb, tt_p)
            # full[e,h] = sum_c wv[e,c] T[h,c]; ctx[e] = full[e, e//HD] + bv
            full_p = epp.tile([C, NH], F32, tag="ep_ps")
            nc.tensor.matmul(full_p, lhsT=wvt_sb, rhs=tt_sb,
                             start=True, stop=True)
            ctx_sb = ep.tile([C, 1], F32)
            for h in range(NH):
                rows = slice(h * HD, (h + 1) * HD)
                nc.scalar.activation(ctx_sb[rows, :], full_p[rows, h:h + 1],
                                     func=mybir.ActivationFunctionType.Identity,
                                     bias=vecs_sb[rows, 0:1])
            o_p = epp.tile([C, 1], F32, tag="ep_ps")
            nc.tensor.matmul(o_p, lhsT=wot_sb, rhs=ctx_sb, start=True, stop=True)
            o_sb = ep.tile([C, 1], F32)
            nc.scalar.activation(o_sb, o_p,
                                 func=mybir.ActivationFunctionType.Identity,
                                 bias=vecs_sb[:, 1:2])
            # LayerNorm over partitions via ones-matmul reductions
            mu_p = epp.tile([1, 1], F32, tag="ep_ps")
            nc.tensor.matmul(mu_p, lhsT=onesf_sb, rhs=o_sb, start=True, stop=True)
            mu_sb = ep.tile([1, 1], F32)
            nc.scalar.activation(mu_sb, mu_p,
                                 func=mybir.ActivationFunctionType.Copy,
                                 scale=1.0 / C)
            mub_p = epp.tile([C, 1], F32, tag="ep_ps")
            nc.tensor.matmul(mub_p, lhsT=onesrow_sb, rhs=mu_sb,
                             start=True, stop=True)
            cent = ep.tile([C, 1], F32)
            nc.vector.tensor_tensor(out=cent, in0=o_sb, in1=mub_p,
                                    op=mybir.AluOpType.subtract)
            sq = ep.tile([C, 1], F32)
            nc.vector.tensor_mul(sq, cent, cent)
            var_p = epp.tile([1, 1], F32, tag="ep_ps")
            nc.tensor.matmul(var_p, lhsT=onesf_sb, rhs=sq, start=True, stop=True)
            sd_sb = ep.tile([1, 1], F32)
            nc.scalar.activation(sd_sb, var_p,
                                 func=mybir.ActivationFunctionType.Sqrt,
                                 bias=eps_sb, scale=1.0 / C)
            rstd = ep.tile([1, 1], F32)
            nc.vector.reciprocal(rstd, sd_sb)
            rstdb_p = epp.tile([C, 1], F32, tag="ep_ps")
            nc.tensor.matmul(rstdb_p, lhsT=onesrow_sb, rhs=rstd,
                             start=True, stop=True)
            t1 = ep.tile([C, 1], F32)
            nc.vector.tensor_mul(t1, cent, rstdb_p)
            ln_sb = ep.tile([C, 1], F32)
            nc.vector.tensor_scalar(out=ln_sb, in0=t1,
                                    scalar1=vecs_sb[:, 2:3],
                                    scalar2=vecs_sb[:, 3:4],
                                    op0=mybir.AluOpType.mult,
                                    op1=mybir.AluOpType.add)

            # ---------------- pass B ----------------
            with tc.tile_pool(name="xf32", bufs=3) as xf_p:
                for t in range(nt):
                    sl = slice(t * tile_w, (t + 1) * tile_w)
                    xf_t = xf_p.tile([C, tile_w], F32)
                    nc.sync.dma_start(out=xf_t, in_=xf_d[:, sl])
                    nc.vector.tensor_scalar_add(out=xf_t, in0=xf_t,
                                                scalar1=ln_sb[:, 0:1])
                    nc.sync.dma_start(out=out_d[:, sl], in_=xf_t)

    return nc


_NC_CACHE = {}


def _get_nc(tok=TOK, tile_w=TILE_W, taylor=True):
    key = (tok, tile_w, taylor)
    if key not in _NC_CACHE:
        _NC_CACHE[key] = _build_nc(tok, tile_w, taylor)
    return _NC_CACHE[key]


def _host_prep(inputs):
    """Compute per-(batch,head) folded query vectors and epilogue constants."""
    emb = np.asarray(inputs["emb"], np.float32)
    domain_idx = np.asarray(inputs["domain_idx"]).astype(np.int64)
    q_proj_w = np.asarray(inputs["q_proj_w"], np.float32)
    q_proj_b = np.asarray(inputs["q_proj_b"], np.float32)
    wq = np.asarray(inputs["wq"], np.float32)
    bq = np.asarray(inputs["bq"], np.float32)
    wk = np.asarray(inputs["wk"], np.float32)
    wv = np.asarray(inputs["wv"], np.float32)
    bv = np.asarray(inputs["bv"], np.float32)
    wo = np.asarray(inputs["wo"], np.float32)
    bo = np.asarray(inputs["bo"], np.float32)
    ln_g = np.asarray(inputs["ln_g"], np.float32)
    ln_b = np.asarray(inputs["ln_b"], np.float32)

    de = emb[domain_idx]                        # (B, E)
    q = de @ q_proj_w.T + q_proj_b
    qh = (q @ wq.T + bq).reshape(B, NH, HD)
    # a[b,h,c] = SCALE * sum_d qh[b,h,d] * wk[h*HD+d, c]
    wk_h = wk.reshape(NH, HD, C)
    a = SCALE * np.einsum("bhd,hdc->bhc", qh, wk_h)   # (B, NH, C)

    # logit magnitude guard (no max-subtraction on device)
    amax = float(np.max(np.linalg.norm(a, axis=-1)))
    taylor = amax * 45.0 < 0.03   # Taylor only when logits provably < 0.03
    if amax * 45.0 > 60.0:
        raise NotImplementedError(
            f"logit bound {amax * 45.0:.1f} too large for exp without "
            "max-subtraction")

    vecs = np.stack([bv, bo, ln_g, ln_b], axis=1).astype(np.float32)
    return a, wv.T.copy(), wo.T.copy(), vecs, taylor


def _make_sel(core):
    """(40, 4) selector: out[h,:] = sum over my pair r of (G[r*5+h] + G[r*5+4])."""
    sel = np.zeros((5 * N_CORES, NH), np.float32)
    b = core // 2
    for r in (2 * b, 2 * b + 1):
        for h in range(NH):
            sel[r * 5 + h, h] = 1.0
            sel[r * 5 + 4, h] = 1.0
    return sel


def kernel(**inputs):
    global LAST_EXEC_NS, LAST_RESULTS, LAST_IN_MAPS, LAST_TAYLOR
    x = np.asarray(inputs["x"], np.float32)
    Bx, Cx, D, H, W = x.shape
    assert (Bx, Cx, D * H * W) == (B, C, N_FULL)
    xr = np.ascontiguousarray(x.reshape(B, C, N_FULL))

    a, wvt, wot, vecs, taylor = _host_prep(inputs)

    ident = np.eye(128, dtype=ml_dtypes.bfloat16)
    identf = np.eye(NH, dtype=np.float32)
    onesf = np.ones((128, 1), np.float32)
    onesrow = np.ones((1, 128), np.float32)

    in_maps = []
    for r in range(N_CORES):
        b, half = r // 2, r % 2
        sl = slice(half * TOK, (half + 1) * TOK)
        xs = np.ascontiguousarray(xr[b, :, sl])
        in_maps.append({
            "xb": xs.astype(ml_dtypes.bfloat16),
            "xf": xs,
            "a_q": np.ascontiguousarray(a[b].T).astype(ml_dtypes.bfloat16),
            "ident": ident,
            "identf": identf,
            "ones_f": onesf,
            "ones_row": onesrow,
            "sel": _make_sel(r),
            "wvt": wvt,
            "wot": wot,
            "vecs": vecs,
        })

    nc = _get_nc(TOK, TILE_W, taylor)
    LAST_IN_MAPS = in_maps
    LAST_TAYLOR = taylor
    res = run_bass_kernel_spmd(nc, in_maps, list(range(N_CORES)), trace=TRACE)
    LAST_EXEC_NS = res.exec_time_ns
    LAST_RESULTS = res

    out = np.empty((B, C, N_FULL), np.float32)
    for r in range(N_CORES):
        b, half = r // 2, r % 2
        out[b, :, half * TOK:(half + 1) * TOK] = res.results[r]["out"]
    return out.reshape(B, C, D, H, W)



# revision 29
# speedup vs baseline: 3.8145x; 3.8145x over previous
"""Trainium2 Bass kernel for nn_CrossDomainAttention (B=4, C=128, D*H*W=131072).

Math reduction (host folds the query chain):
  scores[b,h,n] = scale * qh[b,h] . (wk_h @ x_n + bk_h)  ==  a[b,h] . x_n + const
  softmax is shift-invariant -> drop the const.
  Taylor mode (|logit| < 0.03): attn_n ~ (1 + a.x_n) / (N + a.sum_x), so the
  whole attention reduces to the channel Gram matrix G = X X^T and s = sum x:
    weighted_sum_h = s + G a_h ;  denom_h = N + a_h.s
  ctx[e] = wv[e,:] @ (weighted_sum_{h(e)} / denom_{h(e)}) + bv[e]
  out = wo @ ctx + bo ; ln = LayerNorm(out) ; result = x + ln[:, None]

Device (8 cores SPMD; core r handles batch r//2, token half r%2 = 65536 tok):
  x shard streamed in fp16 ONCE and kept resident in SBUF (128 KiB/partition).
  pass A: per 128-token block: PE transpose, PSUM->SBUF copy (+ones col),
    PE matmul accumulating [G | s] into one (128,129) PSUM tile.
  pair-wise AllGather of [G|s] (cores 2b, 2b+1), epilogue PSUM-accumulates
    the pair, computes ctx/out/LayerNorm -> ln (128,1).
  pass B: from resident fp16 x, tensor_scalar add ln per partition, store
    fp16 out (host upcasts to fp32).
"""

import math
import os
import sys
from contextlib import ExitStack

import numpy as np

if "/opt/trn_rl_repo" not in sys.path:
    sys.path.insert(0, "/opt/trn_rl_repo")

import ml_dtypes

import concourse.bass as bass
import concourse.mybir as mybir
import concourse.tile as tile
from concourse.bass_utils import run_bass_kernel_spmd


def _legalize_sync_waits(bir_json: bytes) -> bytes:
    """This toolchain's walrus supports one sync-wait slot per instruction
    (ISA EVENTS struct). Tile emits instructions with several waits; split
    the extras onto same-engine NoOps placed immediately before."""
    import orjson

    d = orjson.loads(bir_json)
    ctr = 0
    for f in d.get("functions", []):
        for bb in f.get("blocks", []):
            new = []
            changed = False
            for inst in bb.get("instructions", []):
                si = inst.get("sync_info")
                waits = (si or {}).get("on_wait") or []
                if len(waits) > 1:
                    changed = True
                    for w in waits[:-1]:
                        ctr += 1
                        nop = {
                            "engine": inst["engine"],
                            "ins": [],
                            "outs": [],
                            "name": f"legwait-{ctr}",
                            "opcode": "NoOp",
                            "sync_info": {"on_update": [], "on_wait": [w]},
                        }
                        if "debug" in inst:
                            nop["debug"] = inst["debug"]
                        new.append(nop)
                    si["on_wait"] = [waits[-1]]
                new.append(inst)
            if changed:
                bb["instructions"] = new
    return orjson.dumps(d)


def _install_sync_wait_patch():
    import concourse.bass_utils as bu
    import concourse.bass2jax as b2j

    if getattr(bu, "_sync_wait_patch", False):
        return
    orig = bu.compile_bir_kernel

    def patched(bir_json, tmpdir, neff_name="file.neff"):
        return orig(_legalize_sync_waits(bytes(bir_json)), tmpdir, neff_name)

    bu.compile_bir_kernel = patched
    bu._sync_wait_patch = True
    if getattr(b2j, "compile_bir_kernel", None) is orig:
        b2j.compile_bir_kernel = patched


_install_sync_wait_patch()

F32 = mybir.dt.float32
F16 = mybir.dt.float16
F8 = mybir.dt.float8e4
BF16 = mybir.dt.bfloat16

B = 4
C = 128          # embed dim == channel dim
NH = 4           # heads
HD = 32
N_FULL = 32 * 64 * 64   # 131072 tokens per batch
N_CORES = 8
TOK = N_FULL // 2       # tokens per core (65536)
SCALE = 1.0 / math.sqrt(HD)
LN_EPS = 1e-5

# knobs (taylor kernel)
RES_W = 4096             # resident tile width (fp16: 8 KiB/partition each)
CHUNK = 1024             # tokens per transpose/copy chunk
BLK = 128                # tokens per PE block

# knobs (exp fallback kernel)
TILE_W = 4096

# module-level controls for the test harness
TRACE = False
LAST_EXEC_NS = None
LAST_RESULTS = None
LAST_IN_MAPS = None
LAST_TAYLOR = True
LAST_NC = None


def _build_trivial_nc():
    """Passthrough kernel with identical I/O: out = copy(xb). Used by the
    test harness to subtract transfer/dispatch overhead when timing."""
    nc = bass.Bass("TRN2", target_bir_lowering=False, debug=False,
                   num_devices=N_CORES)
    xb_d = nc.dram_tensor("xb", [C, TOK], F16, kind="ExternalInput")
    nc.dram_tensor("a_q", [C, NH], F32, kind="ExternalInput")
    nc.dram_tensor("ident", [128, 128], F16, kind="ExternalInput")
    nc.dram_tensor("hsel", [NH, 128], F32, kind="ExternalInput")
    nc.dram_tensor("ones_f", [128, 1], F32, kind="ExternalInput")
    nc.dram_tensor("ones_row", [1, 128], F32, kind="ExternalInput")
    nc.dram_tensor("wvt", [C, C], F32, kind="ExternalInput")
    nc.dram_tensor("wot", [C, C], F32, kind="ExternalInput")
    nc.dram_tensor("vecs", [C, 4], F32, kind="ExternalInput")
    out_d = nc.dram_tensor("out", [C, TOK], F16, kind="ExternalOutput")
    with tile.TileContext(nc) as tc:
        with tc.tile_pool(name="p", bufs=2) as p:
            t0 = p.tile([C, RES_W], F16)
            nc.sync.dma_start(out=t0, in_=xb_d[:, 0:RES_W])
            nc.sync.dma_start(out=out_d[:, 0:RES_W], in_=t0)
    return nc


def _build_nc_taylor(tok=TOK, res_w=RES_W):
    """Fused single-pass Gram kernel (taylor mode). One core's SPMD program."""
    nt = tok // res_w
    chunks_per_tile = res_w // CHUNK
    blocks_per_chunk = CHUNK // BLK
    total_blocks = tok // BLK

    nc = bass.Bass("TRN2", target_bir_lowering=False, debug=False,
                   num_devices=N_CORES)

    # ---- DRAM I/O ----
    xb_d = nc.dram_tensor("xb", [C, tok], F16, kind="ExternalInput")
    a_d = nc.dram_tensor("a_q", [C, NH], F32, kind="ExternalInput")
    ident_d = nc.dram_tensor("ident", [128, 128], F16, kind="ExternalInput")
    hsel_d = nc.dram_tensor("hsel", [NH, 128], F32, kind="ExternalInput")
    onesf_d = nc.dram_tensor("ones_f", [128, 1], F32, kind="ExternalInput")
    onesrow_d = nc.dram_tensor("ones_row", [1, 128], F32, kind="ExternalInput")
    wvt_d = nc.dram_tensor("wvt", [C, C], F32, kind="ExternalInput")
    wot_d = nc.dram_tensor("wot", [C, C], F32, kind="ExternalInput")
    vecs_d = nc.dram_tensor("vecs", [C, 4], F32, kind="ExternalInput")
    out_d = nc.dram_tensor("out", [C, tok], F16, kind="ExternalOutput")

    with tile.TileContext(nc) as tc, ExitStack() as stack:
        consts = stack.enter_context(tc.tile_pool(name="consts", bufs=1))
        accp = stack.enter_context(
            tc.tile_pool(name="acc", bufs=1, space="PSUM"))
        dramp = stack.enter_context(
            tc.tile_pool(name="dram", bufs=1, space="DRAM"))
        xres_p = stack.enter_context(tc.tile_pool(name="xres", bufs=nt))

        # transpose identity first — everything in pass A depends on it.
        # Small/const DMAs ride the scalar-engine HWDGE ring so the sync
        # ring is dedicated to the big x streams.
        ident_sb = consts.tile([128, 128], F16)
        nc.scalar.dma_start(out=ident_sb, in_=ident_d[:, :])

        # persistent psum accumulator: G (128, 128); s reduced on DVE
        g_acc = accp.tile([128, 128], F32)

        # collective bounce buffers (DRAM): payload [G@a | s] (128, 5)
        cc_in = dramp.tile([128, 5], F32)
        cc_out = dramp.tile([2 * 128, 5], F32)

        # ---------------- load x (resident) ----------------
        # First tiles arrive in small pieces so PE can start ~3 us earlier.
        xres = []
        for t in range(nt):
            xt = xres_p.tile([C, res_w], F16, tag="xres")
            piece = 512 if t == 0 else 1024 if t == 1 else 2048 if t == 2 \
                else res_w
            for p0 in range(0, res_w, piece):
                nc.sync.dma_start(
                    out=xt[:, p0:p0 + piece],
                    in_=xb_d[:, t * res_w + p0: t * res_w + p0 + piece])
            xres.append(xt)

        # epilogue constants (not needed until after pass A)
        a_sb = consts.tile([C, NH], F32)
        nc.scalar.dma_start(out=a_sb, in_=a_d[:, :])
        hsel_sb = consts.tile([NH, 128], F32)
        nc.scalar.dma_start(out=hsel_sb, in_=hsel_d[:, :])
        onesf_sb = consts.tile([128, 1], F32)
        nc.scalar.dma_start(out=onesf_sb, in_=onesf_d[:, :])
        onesrow_sb = consts.tile([1, 128], F32)
        nc.scalar.dma_start(out=onesrow_sb, in_=onesrow_d[:, :])
        wvt_sb = consts.tile([C, C], F32)
        nc.scalar.dma_start(out=wvt_sb, in_=wvt_d[:, :])
        wot_sb = consts.tile([C, C], F32)
        nc.scalar.dma_start(out=wot_sb, in_=wot_d[:, :])
        vecs_sb = consts.tile([C, 4], F32)
        nc.scalar.dma_start(out=vecs_sb, in_=vecs_d[:, :])
        eps_sb = consts.tile([1, 1], F32)
        nc.vector.memset(eps_sb, LN_EPS)
        # preload the Sqrt activation table so the epilogue doesn't pay the
        # table-load latency on the critical path
        sqwarm = consts.tile([1, 1], F32)
        nc.scalar.activation(sqwarm, eps_sb,
                             func=mybir.ActivationFunctionType.Sqrt,
                             bias=eps_sb, scale=1.0)

        # ---------------- pass A: accumulate G (fp8 DoubleRow) + s ----------
        # Per 512-token chunk: 4 PE transposes (fp16), one PSUM->SBUF fp8
        # copy, then 2 DoubleRow gram matmuls (each contracts 256 tokens).
        groups_per_chunk = CHUNK // (2 * BLK)
        total_groups = tok // (2 * BLK)
        spart = consts.tile([128, nt], F32)
        grp_idx = 0
        cidx = 0
        with tc.tile_pool(name="xtp", bufs=4, space="PSUM") as xtp_p, \
             tc.tile_pool(name="xts", bufs=4) as xts_p, \
             tc.tile_pool(name="scr", bufs=2) as scr_p:
            for t in range(nt):
                for ch in range(chunks_per_tile):
                    base = ch * CHUNK
                    xtp = xtp_p.tile([128, CHUNK], F16)
                    for j in range(blocks_per_chunk):
                        xblk = xres[t][:, base + j * BLK: base + (j + 1) * BLK]
                        nc.tensor.transpose(
                            xtp[:, j * BLK:(j + 1) * BLK], xblk, ident_sb)
                    xts = xts_p.tile([128, CHUNK], F8)
                    if cidx % 8 < 3:
                        nc.vector.tensor_copy(out=xts, in_=xtp)
                    else:
                        nc.scalar.copy(out=xts, in_=xtp)
                    cidx += 1
                    for g in range(groups_per_chunk):
                        ap = xts[:, g * 256:(g + 1) * 256].rearrange(
                            "p (two f) -> p two f", two=2)
                        first = grp_idx == 0
                        last = grp_idx == total_groups - 1
                        grp_idx += 1
                        nc.tensor.matmul(
                            g_acc[:, :], lhsT=ap, rhs=ap,
                            start=first, stop=last,
                            perf_mode=mybir.MatmulPerfMode.DoubleRow)
                # exact token-sum from the resident fp16 tile (DVE
                # tensor_scalar runs in the fast mode; accum_out sums the
                # free axis — much cheaper than TensorReduce)
                scr = scr_p.tile([128, res_w], F16)
                nc.vector.tensor_scalar(
                    out=scr, in0=xres[t], scalar1=0.0, scalar2=None,
                    op0=mybir.AluOpType.add, op1=mybir.AluOpType.add,
                    accum_out=spart[:, t:t + 1])

        # ---------------- collective (pair-wise) + epilogue ----------------
        # Pre-reduce on device: payload is [G_local @ a | s_local] (128, 5).
        g_sb = consts.tile([128, 128], F32)
        nc.vector.tensor_copy(out=g_sb, in_=g_acc[:, :])
        pay_sb = consts.tile([128, 5], F32)
        gal_ps = accp.tile([128, NH], F32, tag="gal")
        nc.tensor.matmul(gal_ps, lhsT=g_sb, rhs=a_sb, start=True, stop=True)
        nc.vector.tensor_copy(out=pay_sb[:, 0:4], in_=gal_ps)
        nc.vector.tensor_reduce(out=pay_sb[:, 4:5], in_=spart,
                                axis=mybir.AxisListType.X,
                                op=mybir.AluOpType.add)
        nc.sync.dma_start(out=cc_in[:, :], in_=pay_sb[:, :])
        nc.gpsimd.collective_compute(
            "AllGather",
            mybir.AluOpType.bypass,
            replica_groups=[[2 * b, 2 * b + 1] for b in range(B)],
            ins=[cc_in[:, :].opt()],
            outs=[cc_out[:, :].opt()],
        )

        with tc.tile_pool(name="epi", bufs=1) as ep, \
             tc.tile_pool(name="epp", bufs=2, space="PSUM") as epp:
            g01_sb = ep.tile([128, 2 * 5], F32)
            nc.sync.dma_start(
                out=g01_sb.rearrange("p (k f) -> p k f", f=5),
                in_=cc_out.rearrange("(k p) f -> p k f", p=128))
            # s_tot = s0 + s1 ; numer = (Ga0 + Ga1) + s_tot
            s_tot = ep.tile([128, 1], F32)
            nc.vector.tensor_tensor(out=s_tot, in0=g01_sb[:, 4:5],
                                    in1=g01_sb[:, 9:10],
                                    op=mybir.AluOpType.add)
            numer_sb = ep.tile([128, NH], F32)
            nc.vector.tensor_tensor(out=numer_sb, in0=g01_sb[:, 0:4],
                                    in1=g01_sb[:, 5:9],
                                    op=mybir.AluOpType.add)
            nc.vector.tensor_scalar_add(out=numer_sb, in0=numer_sb,
                                        scalar1=s_tot[:, 0:1])
            # ds[h] = a_h . s_tot ;  inv[h] = 1 / (N + ds[h])
            ds_ps = epp.tile([NH, 1], F32, tag="ep_ps2")
            nc.tensor.matmul(ds_ps, lhsT=a_sb, rhs=s_tot,
                             start=True, stop=True)
            den_sb = ep.tile([NH, 1], F32)
            nc.vector.tensor_scalar_add(out=den_sb, in0=ds_ps,
                                        scalar1=float(N_FULL))
            inv_sb = ep.tile([NH, 1], F32)
            nc.vector.reciprocal(inv_sb, den_sb)
            # invb[e] = inv[head(e)]
            invb_ps = epp.tile([128, 1], F32, tag="ep_ps2")
            nc.tensor.matmul(invb_ps, lhsT=hsel_sb, rhs=inv_sb,
                             start=True, stop=True)
            # W[e,h] = wv[e,:] @ numer[:,h]
            w_ps = epp.tile([128, NH], F32, tag="ep_ps")
            nc.tensor.matmul(w_ps, lhsT=wvt_sb, rhs=numer_sb,
                             start=True, stop=True)
            wsel_sb = ep.tile([128, 1], F32)
            for h in range(NH):
                rows = slice(h * HD, (h + 1) * HD)
                if h % 2 == 0:
                    nc.vector.tensor_copy(out=wsel_sb[rows, :],
                                          in_=w_ps[rows, h:h + 1])
                else:
                    nc.scalar.copy(out=wsel_sb[rows, :],
                                   in_=w_ps[rows, h:h + 1])
            ctx0_sb = ep.tile([128, 1], F32)
            nc.vector.tensor_tensor(out=ctx0_sb, in0=wsel_sb, in1=invb_ps,
                                    op=mybir.AluOpType.mult)
            ctx_sb = ep.tile([128, 1], F32)
            nc.vector.tensor_scalar_add(out=ctx_sb, in0=ctx0_sb,
                                        scalar1=vecs_sb[:, 0:1])
            # wot is host-folded with P = I - J/128, so o_ps is already the
            # mean-centered LayerNorm numerator; vecs col1 = P @ bo.
            o_ps = epp.tile([C, 1], F32, tag="ep_ps")
            nc.tensor.matmul(o_ps, lhsT=wot_sb, rhs=ctx_sb,
                             start=True, stop=True)
            cent = ep.tile([C, 1], F32)
            nc.vector.tensor_scalar_add(out=cent, in0=o_ps,
                                        scalar1=vecs_sb[:, 1:2])
            sq = ep.tile([C, 1], F32)
            nc.vector.tensor_mul(sq, cent, cent)
            var_ps = epp.tile([1, 1], F32, tag="ep_ps2")
            nc.tensor.matmul(var_ps, lhsT=onesf_sb, rhs=sq,
                             start=True, stop=True)
            sd_sb = ep.tile([1, 1], F32)
            nc.scalar.activation(sd_sb, var_ps,
                                 func=mybir.ActivationFunctionType.Sqrt,
                                 bias=eps_sb, scale=1.0 / C)
            rstd = ep.tile([1, 1], F32)
            nc.vector.reciprocal(rstd, sd_sb)
            rstdb_ps = epp.tile([C, 1], F32, tag="ep_ps2")
            nc.tensor.matmul(rstdb_ps, lhsT=onesrow_sb, rhs=rstd,
                             start=True, stop=True)
            t1 = ep.tile([C, 1], F32)
            nc.vector.tensor_mul(t1, cent, rstdb_ps)
            ln_sb = ep.tile([C, 1], F32)
            nc.vector.tensor_scalar(out=ln_sb, in0=t1,
                                    scalar1=vecs_sb[:, 2:3],
                                    scalar2=vecs_sb[:, 3:4],
                                    op0=mybir.AluOpType.mult,
                                    op1=mybir.AluOpType.add)

            # ---------------- pass B: out = x + ln ----------------
            with tc.tile_pool(name="stage", bufs=3) as st_p:
                for t in range(nt):
                    sl = slice(t * res_w, (t + 1) * res_w)
                    stage = st_p.tile([C, res_w], F16)
                    nc.vector.tensor_scalar_add(out=stage, in0=xres[t],
                                                scalar1=ln_sb[:, 0:1])
                    nc.sync.dma_start(out=out_d[:, sl], in_=stage)

    return nc


def _build_nc_exp(tok=TOK, tile_w=TILE_W):
    """Fallback two-pass kernel with on-device exp (non-taylor logits)."""
    nt = tok // tile_w
    chunks_per_tile = tile_w // CHUNK
    blocks_per_chunk = CHUNK // BLK
    total_blocks = tok // BLK

    nc = bass.Bass("TRN2", target_bir_lowering=False, debug=False,
                   num_devices=N_CORES)

    # ---- DRAM I/O ----
    xb_d = nc.dram_tensor("xb", [C, tok], BF16, kind="ExternalInput")
    xf_d = nc.dram_tensor("xf", [C, tok], F32, kind="ExternalInput")
    a_d = nc.dram_tensor("a_q", [C, NH], BF16, kind="ExternalInput")
    ident_d = nc.dram_tensor("ident", [128, 128], BF16, kind="ExternalInput")
    identf_d = nc.dram_tensor("identf", [NH, NH], F32, kind="ExternalInput")
    onesf_d = nc.dram_tensor("ones_f", [128, 1], F32, kind="ExternalInput")
    onesrow_d = nc.dram_tensor("ones_row", [1, 128], F32, kind="ExternalInput")
    sel_d = nc.dram_tensor("sel", [5 * N_CORES, NH], F32, kind="ExternalInput")
    wvt_d = nc.dram_tensor("wvt", [C, C], F32, kind="ExternalInput")
    wot_d = nc.dram_tensor("wot", [C, C], F32, kind="ExternalInput")
    vecs_d = nc.dram_tensor("vecs", [C, 4], F32, kind="ExternalInput")
    out_d = nc.dram_tensor("out", [C, tok], F32, kind="ExternalOutput")

    with tile.TileContext(nc) as tc, ExitStack() as stack:
        consts = stack.enter_context(tc.tile_pool(name="consts", bufs=1))
        accp = stack.enter_context(
            tc.tile_pool(name="acc", bufs=1, space="PSUM"))
        dramp = stack.enter_context(
            tc.tile_pool(name="dram", bufs=1, space="DRAM"))

        # constants into SBUF
        a_sb = consts.tile([C, NH], BF16)
        nc.sync.dma_start(out=a_sb, in_=a_d[:, :])
        ident_sb = consts.tile([128, 128], BF16)
        nc.sync.dma_start(out=ident_sb, in_=ident_d[:, :])
        identf_sb = consts.tile([NH, NH], F32)
        nc.sync.dma_start(out=identf_sb, in_=identf_d[:, :])
        onesf_sb = consts.tile([128, 1], F32)
        nc.sync.dma_start(out=onesf_sb, in_=onesf_d[:, :])
        onesrow_sb = consts.tile([1, 128], F32)
        nc.sync.dma_start(out=onesrow_sb, in_=onesrow_d[:, :])
        sel_sb = consts.tile([5 * N_CORES, NH], F32)
        nc.sync.dma_start(out=sel_sb, in_=sel_d[:, :])
        wvt_sb = consts.tile([C, C], F32)
        nc.sync.dma_start(out=wvt_sb, in_=wvt_d[:, :])
        wot_sb = consts.tile([C, C], F32)
        nc.sync.dma_start(out=wot_sb, in_=wot_d[:, :])
        vecs_sb = consts.tile([C, 4], F32)
        nc.sync.dma_start(out=vecs_sb, in_=vecs_d[:, :])
        eps_sb = consts.tile([1, 1], F32)
        nc.vector.memset(eps_sb, LN_EPS)
        zeros_sb = consts.tile([128, 1], F32)
        nc.vector.memset(zeros_sb, 0.0)

        # persistent psum accumulator: rows 0..3 = [sum w'x | sum w']_h,
        # row 4 = [sum x | count]
        s_acc = accp.tile([5, 129], F32)

        # collective bounce buffers (DRAM)
        cc_in = dramp.tile([5, 129], F32)
        cc_out = dramp.tile([5 * N_CORES, 129], F32, addr_space="Shared")

        # ---------------- pass A ----------------
        blk_idx = 0
        with tc.tile_pool(name="xbf", bufs=3) as xbf_p, \
             tc.tile_pool(name="lg", bufs=2, space="PSUM") as lg_p, \
             tc.tile_pool(name="xtp", bufs=2, space="PSUM") as xtp_p, \
             tc.tile_pool(name="wtile", bufs=3) as w_p, \
             tc.tile_pool(name="xts", bufs=3) as xts_p:
            for t in range(nt):
                xt = xbf_p.tile([C, tile_w], BF16)
                nc.sync.dma_start(out=xt, in_=xb_d[:, t * tile_w:(t + 1) * tile_w])
                for ch in range(chunks_per_tile):
                    base = ch * CHUNK
                    lg = lg_p.tile([128, 4 * blocks_per_chunk], F32)
                    xtp = xtp_p.tile([128, CHUNK], F32)
                    for j in range(blocks_per_chunk):
                        xblk = xt[:, base + j * BLK: base + (j + 1) * BLK]
                        nc.tensor.matmul(lg[:, j * 4:(j + 1) * 4], lhsT=xblk,
                                         rhs=a_sb, start=True, stop=True)
                        nc.tensor.matmul(xtp[:, j * BLK:(j + 1) * BLK],
                                         lhsT=xblk, rhs=ident_sb,
                                         start=True, stop=True)
                    # w' tile: (128, 5*blocks): cols j*5+0..3 = w', j*5+4 = 1
                    wt = w_p.tile([128, 5 * blocks_per_chunk], BF16)
                    wt3 = wt.rearrange("p (j f) -> p j f", f=5)
                    nc.vector.memset(wt3[:, :, 4:5], 1.0)
                    lg3 = lg.rearrange("p (j f) -> p j f", f=4)
                    escr = w_p.tile([128, 4 * blocks_per_chunk], F32,
                                    tag="escr")
                    nc.scalar.activation(out=escr, in_=lg,
                                         func=mybir.ActivationFunctionType.Exp,
                                         bias=zeros_sb)
                    es3 = escr.rearrange("p (j f) -> p j f", f=4)
                    nc.vector.tensor_scalar(
                        out=wt3[:, :, 0:4], in0=es3, scalar1=-1.0,
                        scalar2=None, op0=mybir.AluOpType.add)
                    # xts: (128, 129*blocks): per block 128 cols xT + ones col
                    xts = xts_p.tile([128, 129 * blocks_per_chunk], BF16)
                    xts3 = xts.rearrange("p (j f) -> p j f", f=129)
                    nc.vector.memset(xts3[:, :, 128:129], 1.0)
                    if ch % 2 == 0:
                        nc.vector.tensor_copy(out=xts3[:, :, 0:128],
                                              in_=xtp.rearrange(
                                                  "p (j f) -> p j f", f=BLK))
                    else:
                        nc.scalar.copy(out=xts3[:, :, 0:128],
                                       in_=xtp.rearrange(
                                           "p (j f) -> p j f", f=BLK))
                    for j in range(blocks_per_chunk):
                        first = blk_idx == 0
                        last = blk_idx == total_blocks - 1
                        blk_idx += 1
                        nc.tensor.matmul(
                            s_acc[:, :],
                            lhsT=wt[:, j * 5:(j + 1) * 5],
                            rhs=xts[:, j * 129:(j + 1) * 129],
                            start=first, stop=last)

        # ---------------- collective + epilogue ----------------
        s_sb = consts.tile([5, 129], F32)
        nc.vector.tensor_copy(out=s_sb, in_=s_acc[:, :])
        nc.sync.dma_start(out=cc_in[:, :], in_=s_sb[:, :])
        nc.gpsimd.collective_compute(
            "AllGather",
            mybir.AluOpType.bypass,
            replica_groups=[list(range(N_CORES))],
            ins=[cc_in[:, :].opt()],
            outs=[cc_out[:, :].opt()],
        )

        with tc.tile_pool(name="epi", bufs=1) as ep, \
             tc.tile_pool(name="epp", bufs=2, space="PSUM") as epp:
            g_sb = ep.tile([5 * N_CORES, 129], F32)
            nc.sync.dma_start(out=g_sb, in_=cc_out[:, :])
            # comb[h,:] = sum over my pair of (S'_h + S'_ones) rows
            comb = epp.tile([NH, 129], F32, tag="ep_ps")
            nc.tensor.matmul(comb, lhsT=sel_sb, rhs=g_sb, start=True, stop=True)
            inv = ep.tile([NH, 1], F32)
            nc.vector.reciprocal(inv, comb[:, 128:129])
            t_sb = ep.tile([NH, C], F32)
            nc.vector.tensor_scalar_mul(t_sb, comb[:, 0:128], inv)
            # transpose T (4,128) -> (128,4)
            tt_p = epp.tile([C, NH], F32, tag="ep_ps")
            nc.tensor.transpose(tt_p, t_sb, identf_sb)
            tt_sb = ep.tile([C, NH], F32)
            nc.vector.tensor_copy(tt_sb, tt_p)
            # full[e,h] = sum_c wv[e,c] T[h,c]; ctx[e] = full[e, e//HD] + bv
            full_p = epp.tile([C, NH], F32, tag="ep_ps")
            nc.tensor.matmul(full_p, lhsT=wvt_sb, rhs=tt_sb,
                             start=True, stop=True)
            ctx_sb = ep.tile([C, 1], F32)
            for h in range(NH):
                rows = slice(h * HD, (h + 1) * HD)
                nc.scalar.activation(ctx_sb[rows, :], full_p[rows, h:h + 1],
                                     func=mybir.ActivationFunctionType.Identity,
                                     bias=vecs_sb[rows, 0:1])
            o_p = epp.tile([C, 1], F32, tag="ep_ps")
            nc.tensor.matmul(o_p, lhsT=wot_sb, rhs=ctx_sb, start=True, stop=True)
            o_sb = ep.tile([C, 1], F32)
            nc.scalar.activation(o_sb, o_p,
                                 func=mybir.ActivationFunctionType.Identity,
                                 bias=vecs_sb[:, 1:2])
            # LayerNorm over partitions via ones-matmul reductions
            mu_p = epp.tile([1, 1], F32, tag="ep_ps")
            nc.tensor.matmul(mu_p, lhsT=onesf_sb, rhs=o_sb, start=True, stop=True)
            mu_sb = ep.tile([1, 1], F32)
            nc.scalar.activation(mu_sb, mu_p,
                                 func=mybir.ActivationFunctionType.Copy,
                                 scale=1.0 / C)
            mub_p = epp.tile([C, 1], F32, tag="ep_ps")
            nc.tensor.matmul(mub_p, lhsT=onesrow_sb, rhs=mu_sb,
                             start=True, stop=True)
            cent = ep.tile([C, 1], F32)
            nc.vector.tensor_tensor(out=cent, in0=o_sb, in1=mub_p,
                                    op=mybir.AluOpType.subtract)
            sq = ep.tile([C, 1], F32)
            nc.vector.tensor_mul(sq, cent, cent)
            var_p = epp.tile([1, 1], F32, tag="ep_ps")
            nc.tensor.matmul(var_p, lhsT=onesf_sb, rhs=sq, start=True, stop=True)
            sd_sb = ep.tile([1, 1], F32)
            nc.scalar.activation(sd_sb, var_p,
                                 func=mybir.ActivationFunctionType.Sqrt,
                                 bias=eps_sb, scale=1.0 / C)
            rstd = ep.tile([1, 1], F32)
            nc.vector.reciprocal(rstd, sd_sb)
            rstdb_p = epp.tile([C, 1], F32, tag="ep_ps")
            nc.tensor.matmul(rstdb_p, lhsT=onesrow_sb, rhs=rstd,
                             start=True, stop=True)
            t1 = ep.tile([C, 1], F32)
            nc.vector.tensor_mul(t1, cent, rstdb_p)
            ln_sb = ep.tile([C, 1], F32)
            nc.vector.tensor_scalar(out=ln_sb, in0=t1,
                                    scalar1=vecs_sb[:, 2:3],
                                    scalar2=vecs_sb[:, 3:4],
                                    op0=mybir.AluOpType.mult,
                                    op1=mybir.AluOpType.add)

            # ---------------- pass B ----------------
            with tc.tile_pool(name="xf32", bufs=3) as xf_p:
                for t in range(nt):
                    sl = slice(t * tile_w, (t + 1) * tile_w)
                    xf_t = xf_p.tile([C, tile_w], F32)
                    nc.sync.dma_start(out=xf_t, in_=xf_d[:, sl])
                    nc.vector.tensor_scalar_add(out=xf_t, in0=xf_t,
                                                scalar1=ln_sb[:, 0:1])
                    nc.sync.dma_start(out=out_d[:, sl], in_=xf_t)

    return nc


_NC_CACHE = {}


def _get_nc(taylor=True):
    key = ("taylor" if taylor else "exp")
    if key not in _NC_CACHE:
        _NC_CACHE[key] = (_build_nc_taylor() if taylor
                          else _build_nc_exp())
    return _NC_CACHE[key]


def _host_prep(inputs):
    """Compute per-(batch,head) folded query vectors and epilogue constants."""
    emb = np.asarray(inputs["emb"], np.float32)
    domain_idx = np.asarray(inputs["domain_idx"]).astype(np.int64)
    q_proj_w = np.asarray(inputs["q_proj_w"], np.float32)
    q_proj_b = np.asarray(inputs["q_proj_b"], np.float32)
    wq = np.asarray(inputs["wq"], np.float32)
    bq = np.asarray(inputs["bq"], np.float32)
    wk = np.asarray(inputs["wk"], np.float32)
    wv = np.asarray(inputs["wv"], np.float32)
    bv = np.asarray(inputs["bv"], np.float32)
    wo = np.asarray(inputs["wo"], np.float32)
    bo = np.asarray(inputs["bo"], np.float32)
    ln_g = np.asarray(inputs["ln_g"], np.float32)
    ln_b = np.asarray(inputs["ln_b"], np.float32)

    de = emb[domain_idx]                        # (B, E)
    q = de @ q_proj_w.T + q_proj_b
    qh = (q @ wq.T + bq).reshape(B, NH, HD)
    # a[b,h,c] = SCALE * sum_d qh[b,h,d] * wk[h*HD+d, c]
    wk_h = wk.reshape(NH, HD, C)
    a = SCALE * np.einsum("bhd,hdc->bhc", qh, wk_h)   # (B, NH, C)

    # logit magnitude guard (no max-subtraction on device)
    amax = float(np.max(np.linalg.norm(a, axis=-1)))
    taylor = amax * 45.0 < 0.03   # Taylor only when logits provably < 0.03
    if amax * 45.0 > 60.0:
        raise NotImplementedError(
            f"logit bound {amax * 45.0:.1f} too large for exp without "
            "max-subtraction")

    vecs = np.stack([bv, bo, ln_g, ln_b], axis=1).astype(np.float32)
    return a, wv.T.copy(), wo.T.copy(), vecs, taylor


def _make_sel(core):
    """(40, 4) selector: out[h,:] = sum over my pair r of (G[r*5+h] + G[r*5+4])."""
    sel = np.zeros((5 * N_CORES, NH), np.float32)
    b = core // 2
    for r in (2 * b, 2 * b + 1):
        for h in range(NH):
            sel[r * 5 + h, h] = 1.0
            sel[r * 5 + 4, h] = 1.0
    return sel


def prepare(inputs):
    """Build (nc, in_maps, assemble) — shared by kernel() and sim harnesses."""
    global LAST_TAYLOR, LAST_NC
    x = np.asarray(inputs["x"], np.float32)
    Bx, Cx, D, H, W = x.shape
    assert (Bx, Cx, D * H * W) == (B, C, N_FULL)
    xr = np.ascontiguousarray(x.reshape(B, C, N_FULL))

    a, wvt, wot, vecs, taylor = _host_prep(inputs)
    LAST_TAYLOR = taylor

    in_maps = []
    if taylor:
        ident = np.eye(128, dtype=np.float16)
        hsel = np.zeros((NH, 128), np.float32)
        for h in range(NH):
            hsel[h, h * HD:(h + 1) * HD] = 1.0
        onesf = np.ones((128, 1), np.float32)
        onesrow = np.ones((1, 128), np.float32)
        # fold the LayerNorm mean-centering projection P = I - J/128 into
        # wot (device then produces the centered numerator directly)
        wot = wot - wot.mean(axis=1, keepdims=True)
        vecs = vecs.copy()
        vecs[:, 1] -= vecs[:, 1].mean()
        for r in range(N_CORES):
            b, half = r // 2, r % 2
            sl = slice(half * TOK, (half + 1) * TOK)
            xs = np.ascontiguousarray(xr[b, :, sl])
            in_maps.append({
                "xb": xs.astype(np.float16),
                "a_q": np.ascontiguousarray(a[b].T).astype(np.float32),
                "ident": ident,
                "hsel": hsel,
                "ones_f": onesf,
                "ones_row": onesrow,
                "wvt": wvt,
                "wot": wot,
                "vecs": vecs,
            })
    else:
        ident = np.eye(128, dtype=ml_dtypes.bfloat16)
        identf = np.eye(NH, dtype=np.float32)
        onesf = np.ones((128, 1), np.float32)
        onesrow = np.ones((1, 128), np.float32)
        for r in range(N_CORES):
            b, half = r // 2, r % 2
            sl = slice(half * TOK, (half + 1) * TOK)
            xs = np.ascontiguousarray(xr[b, :, sl])
            in_maps.append({
                "xb": xs.astype(ml_dtypes.bfloat16),
                "xf": xs,
                "a_q": np.ascontiguousarray(a[b].T).astype(ml_dtypes.bfloat16),
                "ident": ident,
                "identf": identf,
                "ones_f": onesf,
                "ones_row": onesrow,
                "sel": _make_sel(r),
                "wvt": wvt,
                "wot": wot,
                "vecs": vecs,
            })

    nc = _get_nc(taylor)
    LAST_NC = nc

    def assemble(results):
        out = np.empty((B, C, N_FULL), np.float32)
        for r in range(N_CORES):
            b, half = r // 2, r % 2
            out[b, :, half * TOK:(half + 1) * TOK] = np.asarray(
                results[r]["out"]).astype(np.float32)
        return out.reshape(B, C, D, H, W)

    return nc, in_maps, assemble


def kernel(**inputs):
    global LAST_EXEC_NS, LAST_RESULTS, LAST_IN_MAPS
    nc, in_maps, assemble = prepare(inputs)
    LAST_IN_MAPS = in_maps
    res = run_bass_kernel_spmd(nc, in_maps, list(range(N_CORES)), trace=TRACE)
    LAST_EXEC_NS = res.exec_time_ns
    LAST_RESULTS = res
    return assemble(res.results)
